# revision 32
# baseline (speedup 1.0000x reference)
"""ComplexPolarAttention Trainium2 kernel.

score_ij = sum_d mag_i,d mag_j,d cos(phase_i,d - phase_j,d)
         = a_i . a_j + b_i . b_j          with a = mag*cos(phase), b = mag*sin(phase)
out_mag   = softmax(score, axis=1) @ mag
out_phase = softmax(score, axis=1) @ phase

Strategy (8 NeuronCores, SPMD, no collectives):
  - Rows (queries) sharded; keys replicated. Per-core inputs are ROTATED
    along the key axis so that core c's queries are always columns 0..q of
    its own key panel (softmax over keys is permutation invariant), so the
    query operand is just a slice of the key panel.
  - The packed ab^T = [a|b]^T [128=2D, N] panel (host-prepped layout) fuses
    the two score GEMMs into ONE K=128 fp32r matmul per key block of 128.
  - Scores are computed transposed, S^T[k_blk=128, q] in PSUM (one wide
    [128, 1024] exp per key block amortizes ACT's ~352-cycle instruction
    overhead; scores are bounded by D=64 < 88 so unnormalized exp cannot
    overflow), then used as the MOVING operand of the value matmuls whose
    stationary operands are natural-layout [mag|ones] (the ones column
    yields the softmax denominator for free) and phase.
  - PSUM accumulates the numerators over all 64 key blocks; the final
    divide by the denominator happens on host during the gather.
  - All DRAM inputs are chunk-major so every dma_start reads one fully
    contiguous block; the ab^T chunks ride the sync HWDGE queue, the value
    matrices the gpsimd SWDGE queue, so the k-loop's critical first chunk
    lands as early as possible and later chunks stream in under compute.
"""

import ml_dtypes
import numpy as np
from contextlib import ExitStack

import concourse.bass as bass
import concourse.tile as tile
from concourse import bacc, mybir
from concourse.bass_utils import run_bass_kernel_spmd

F32 = mybir.dt.float32
F32R = mybir.dt.float32r
BF16 = mybir.dt.bfloat16
F16 = mybir.dt.float16


def abt_chunk_widths(n):
    widths, rem = [], n
    for w in (512, 512, 1024, 2048, 4096):
        if rem == 0:
            break
        w = min(w, rem)
        widths.append(w)
        rem -= w
    while rem:
        w = min(4096, rem)
        widths.append(w)
        rem -= w
    return widths


def build_program(n=8192, d=64, n_cores=8, enable_asserts=False):
    """Build the SPMD Bass program. Every core runs identical IR; per-core
    behavior comes only from per-core (rotated) input data."""
    assert d == 64
    q = n // n_cores            # queries per core
    kblocks = n // 128          # key blocks of 128
    qblk = q // 2               # half processed per matmul (fp32 moving max 512)
    assert qblk <= 512 and n % 128 == 0

    nc = bacc.Bacc(
        "TRN2",
        target_bir_lowering=False,
        debug=False,
        enable_asserts=enable_asserts,
        num_devices=n_cores,
    )

    # ---- DRAM I/O (all per-core arrays rotated so queries = keys[0:q]) ----
    chunks = abt_chunk_widths(n)
    vchunk = max(1, kblocks // 16)
    nvch = kblocks // vchunk
    # packed [a|b]^T panel, one DRAM tensor per (graded) chunk: the first
    # chunks are small so the k-loop can start ASAP, later ones large so
    # per-partition DMA descriptors amortize
    abt_in = [nc.dram_tensor(f"abt{i}", [128, w], F32R,
                             kind="ExternalInput").ap()
              for i, w in enumerate(chunks)]
    # [mag | ones] value matrix, chunk-major [nvch, 128, vchunk*65]
    mo = nc.dram_tensor("mo", [nvch, 128, vchunk * 65], F16,
                        kind="ExternalInput").ap()
    # phase value matrix, chunk-major [nvch, 128, vchunk*64]
    pv = nc.dram_tensor("pv", [nvch, 128, vchunk * d], F16,
                        kind="ExternalInput").ap()

    negc = nc.dram_tensor("negc", [128, 1], F32, kind="ExternalInput").ap()
    om = nc.dram_tensor("om", [65, q], F32, kind="ExternalOutput").ap()
    op = nc.dram_tensor("op", [2 * d, qblk], F32, kind="ExternalOutput").ap()

    with tile.TileContext(nc) as tc, ExitStack() as ctx:
        const = ctx.enter_context(tc.tile_pool(name="const", bufs=1))
        persist = ctx.enter_context(tc.tile_pool(name="persist", bufs=1))
        epool = ctx.enter_context(tc.tile_pool(name="exps", bufs=6))
        opool = ctx.enter_context(tc.tile_pool(name="outs", bufs=2))
        spool = ctx.enter_context(tc.tile_pool(name="scores", bufs=2, space="PSUM"))
        apool = ctx.enter_context(tc.tile_pool(name="accum", bufs=1, space="PSUM"))

        abt = persist.tile([128, n], F32R)       # [a|b]^T for all keys
        mo_t = persist.tile([128, kblocks, 65], F16)
        pv_t = persist.tile([128, kblocks, d], F16)

        # ab^T chunks on the sync queue -- chunk 0 gates the first matmul
        off = 0
        for i, w in enumerate(chunks):
            nc.sync.dma_start(out=abt[:, off:off + w], in_=abt_in[i])
            off += w
        abq = abt[:, 0:q]          # queries are the first q key columns

        # -C: exp(s - C) so es fits fp16; C = max_i ||v_i||^2 - 10.5 bounds
        # every score via Cauchy-Schwarz, and the shift cancels in softmax
        negc_t = const.tile([128, 1], F32)
        nc.sync.dma_start(out=negc_t[:, :], in_=negc)

        # value matrices on the gpsimd queue, fine-grained and interleaved
        # so the first key blocks' stationaries land just after exp0
        for vi in range(nvch):
            b0 = vi * vchunk
            b1 = b0 + vchunk
            nc.gpsimd.dma_start(out=mo_t[:, b0:b1, :], in_=mo[vi, :, :])
            nc.gpsimd.dma_start(out=pv_t[:, b0:b1, :], in_=pv[vi, :, :])

        # ---- main loop: all q (both 512-halves) in flight at once.
        # Per key block kb: one stationary load of abt_k shared by the two
        # score matmuls (q halves), ONE wide [128, q] exp, value matmuls two
        # key blocks behind (their es-ready semaphores are then already
        # satisfied when the weight loads issue).
        # PSUM budget: scores [128,1024]x2bufs = 4 banks, psA/psB = 4 banks.
        assert q == 2 * qblk
        # one accumulator tile (= one PSUM bank / zero-region) per q half
        psA = [apool.tile([65, qblk], F32, name=f"psA{j}", tag=f"psA{j}")
               for j in range(2)]
        # phase numerators: ONE [128, qblk] bank; the two q halves go to
        # partition halves 0-63 / 64-127 via PE column-group tiling, so the
        # two pv matmuls execute CONCURRENTLY on disjoint column groups
        psB = apool.tile([128, qblk], F32)

        def value_mms(es, kb, first, last):
            for j in range(2):
                nc.tensor.matmul(
                    out=psA[j][:, :],
                    lhsT=mo_t[:, kb, :],
                    rhs=es[:, j * qblk:(j + 1) * qblk],
                    start=first, stop=last,
                )
            for j in range(2):
                nc.tensor.matmul(
                    out=psB[j * d:(j + 1) * d, :],
                    lhsT=pv_t[:, kb, :],
                    rhs=es[:, j * qblk:(j + 1) * qblk],
                    start=first, stop=last,
                    skip_group_check=(j == 1),
                )

        es_hist = []
        for kb in range(kblocks):
            ss = spool.tile([128, q], F32)
            for j in range(2):
                nc.tensor.matmul(
                    out=ss[:, j * qblk:(j + 1) * qblk],
                    lhsT=abt[:, kb * 128:(kb + 1) * 128],
                    rhs=abq[:, j * qblk:(j + 1) * qblk],
                    start=True, stop=True,
                )
            es = epool.tile([128, q], F16)
            nc.scalar.activation(
                es[:, :], ss[:, :], mybir.ActivationFunctionType.Exp,
                bias=negc_t[:, :],
            )
            if len(es_hist) >= 2:
                value_mms(es_hist[-2], kb - 2, first=(kb == 2), last=False)
            es_hist.append(es)
        value_mms(es_hist[-2], kblocks - 2, first=False, last=False)
        value_mms(es_hist[-1], kblocks - 1, first=False, last=True)

        # outputs: PSUM -> SBUF -> DRAM; copies split across DVE and ACT,
        # store DMAs split across the sync and gpsimd queues
        for j in range(2):
            qsl = slice(j * qblk, (j + 1) * qblk)
            oA = opool.tile([65, qblk], F32, tag="oA")
            nc.vector.tensor_copy(oA[:, :], psA[j][:, :])
            nc.sync.dma_start(out=om[:, qsl], in_=oA[:, :])
        oB = opool.tile([128, qblk], F32, tag="oB")
        nc.scalar.copy(oB[:, :], psB[:, :])
        nc.gpsimd.dma_start(out=op[:, :], in_=oB[:, :])

    nc.compile()
    return nc


def make_inputs(mag, phase, n_cores=8):
    """Host-side sharding/layout prep -> per-core (key-rotated) input maps."""
    n, d = mag.shape
    q = n // n_cores
    kblocks = n // 128
    mag = np.ascontiguousarray(mag, dtype=np.float32)
    phase = np.ascontiguousarray(phase, dtype=np.float32)

    a = mag * np.cos(phase)
    b = mag * np.sin(phase)
    cshift = float((mag.astype(np.float64) ** 2).sum(axis=1).max()) - 10.5
    negc_arr = np.full((128, 1), -cshift, np.float32)
    abt_g = np.concatenate([a.T, b.T], axis=0).astype(np.float32)  # [128, n]
    mo_nat = np.concatenate([mag, np.ones((n, 1), np.float32)], axis=1)

    chunks = abt_chunk_widths(n)
    vchunk = max(1, kblocks // 16)
    nvch = kblocks // vchunk

    def tile_nat(x):  # [n, m] -> [nvch, 128, vchunk*m] chunk-major
        m = x.shape[1]
        y = x.reshape(nvch, vchunk, 128, m).transpose(0, 2, 1, 3)
        return np.ascontiguousarray(y.reshape(nvch, 128, vchunk * m))

    in_maps = []
    for c in range(n_cores):
        r = c * q
        abt_c = np.roll(abt_g, -r, axis=1)
        m = {
            "mo": tile_nat(np.roll(mo_nat, -r, axis=0)).astype(np.float16),
            "pv": tile_nat(np.roll(phase, -r, axis=0)).astype(np.float16),
        }
        m["negc"] = negc_arr
        off = 0
        for i, w in enumerate(chunks):
            m[f"abt{i}"] = np.ascontiguousarray(abt_c[:, off:off + w])
            off += w
        in_maps.append(m)
    return in_maps


def gather_outputs(results, n, d, n_cores=8):
    """Per-core [65,q]/[64,q] transposed unnormalized sums -> full outputs."""
    new_mag = np.empty((n, d), np.float32)
    new_phase = np.empty((n, d), np.float32)
    q = n // n_cores
    qblk = q // 2
    for c in range(n_cores):
        om = results[c]["om"]          # [65, q]
        op = results[c]["op"]          # [128, qblk]: q-halves stacked
        den = om[64:65, :]             # [1, q]
        qsl = slice(c * q, (c + 1) * q)
        new_mag[qsl] = (om[:64, :] / den).T
        oph = np.concatenate([op[:d, :], op[d:, :]], axis=1)   # [64, q]
        new_phase[qsl] = (oph / den).T
    return new_mag, new_phase


_PROGRAM_CACHE = {}


def _get_program(n, d, n_cores):
    key = (n, d, n_cores)
    if key not in _PROGRAM_CACHE:
        _PROGRAM_CACHE[key] = build_program(n=n, d=d, n_cores=n_cores)
    return _PROGRAM_CACHE[key]


def kernel(mag, phase):
    mag = np.asarray(mag, dtype=np.float32)
    phase = np.asarray(phase, dtype=np.float32)
    n, d = mag.shape
    n_cores = 8
    nc = _get_program(n, d, n_cores)
    in_maps = make_inputs(mag, phase, n_cores=n_cores)
    res = run_bass_kernel_spmd(nc, in_maps, list(range(n_cores)))
    return gather_outputs(res.results, n, d, n_cores=n_cores)


# revision 33
# speedup vs baseline: 1.1616x; 1.1616x over previous
"""ComplexPolarAttention Trainium2 kernel.

score_ij = sum_d mag_i,d mag_j,d cos(phase_i,d - phase_j,d)
         = a_i . a_j + b_i . b_j          with a = mag*cos(phase), b = mag*sin(phase)
out_mag   = softmax(score, axis=1) @ mag
out_phase = softmax(score, axis=1) @ phase

Strategy (8 NeuronCores, SPMD, no collectives):
  - Rows (queries) sharded; keys replicated. Per-core inputs are ROTATED
    along the key axis so that core c's queries are always columns 0..q of
    its own key panel (softmax over keys is permutation invariant), so the
    query operand is just a slice of the key panel.
  - The packed ab^T = [a|b]^T [128=2D, N] panel (host-prepped layout) fuses
    the two score GEMMs into ONE K=128 fp32r matmul per key block of 128.
  - Scores are computed transposed, S^T[k_blk=128, q] in PSUM (one wide
    [128, 1024] exp per key block amortizes ACT's ~352-cycle instruction
    overhead; scores are bounded by D=64 < 88 so unnormalized exp cannot
    overflow), then used as the MOVING operand of the value matmuls whose
    stationary operands are natural-layout [mag|ones] (the ones column
    yields the softmax denominator for free) and phase.
  - PSUM accumulates the numerators over all 64 key blocks; the final
    divide by the denominator happens on host during the gather.
  - All DRAM inputs are chunk-major so every dma_start reads one fully
    contiguous block; the ab^T chunks ride the sync HWDGE queue, the value
    matrices the gpsimd SWDGE queue, so the k-loop's critical first chunk
    lands as early as possible and later chunks stream in under compute.
"""

import numpy as np
from contextlib import ExitStack

import concourse.bass as bass
import concourse.tile as tile
from concourse import bacc, mybir
from concourse.bass_utils import run_bass_kernel_spmd

F32 = mybir.dt.float32
F32R = mybir.dt.float32r


def build_program(n=8192, d=64, n_cores=8, enable_asserts=False):
    """Build the SPMD Bass program. Every core runs identical IR; per-core
    behavior comes only from per-core (rotated) input data."""
    assert d == 64
    q = n // n_cores            # queries per core
    kblocks = n // 128          # key blocks of 128
    qblk = q // 2               # half processed per matmul (fp32 moving max 512)
    assert qblk <= 512 and n % 128 == 0

    nc = bacc.Bacc(
        "TRN2",
        target_bir_lowering=False,
        debug=False,
        enable_asserts=enable_asserts,
        num_devices=n_cores,
    )

    # ---- DRAM I/O (all per-core arrays rotated so queries = keys[0:q]) ----
    abchunk = min(1024, n)
    nabch = n // abchunk
    vchunk = max(1, kblocks // 16)
    nvch = kblocks // vchunk
    # packed [a|b]^T panel, chunk-major [nabch, 128, abchunk]
    abt_in = nc.dram_tensor("abt", [nabch, 128, abchunk], F32R,
                            kind="ExternalInput").ap()
    # [mag | ones] value matrix, chunk-major [nvch, 128, vchunk*65]
    mo = nc.dram_tensor("mo", [nvch, 128, vchunk * 65], F32R,
                        kind="ExternalInput").ap()
    # phase value matrix, chunk-major [nvch, 128, vchunk*64]
    pv = nc.dram_tensor("pv", [nvch, 128, vchunk * d], F32R,
                        kind="ExternalInput").ap()

    om = nc.dram_tensor("om", [65, q], F32, kind="ExternalOutput").ap()
    op = nc.dram_tensor("op", [d, q], F32, kind="ExternalOutput").ap()

    with tile.TileContext(nc) as tc, ExitStack() as ctx:
        persist = ctx.enter_context(tc.tile_pool(name="persist", bufs=1))
        epool = ctx.enter_context(tc.tile_pool(name="exps", bufs=6))
        opool = ctx.enter_context(tc.tile_pool(name="outs", bufs=2))
        spool = ctx.enter_context(tc.tile_pool(name="scores", bufs=2, space="PSUM"))
        apool = ctx.enter_context(tc.tile_pool(name="accum", bufs=1, space="PSUM"))

        abt = persist.tile([128, n], F32R)       # [a|b]^T for all keys
        mo_t = persist.tile([128, kblocks, 65], F32R)
        pv_t = persist.tile([128, kblocks, d], F32R)

        # ab^T chunks on the sync queue -- chunk 0 (== the query slice)
        # gates the first score matmul and exp
        for ci in range(nabch):
            nc.sync.dma_start(
                out=abt[:, ci * abchunk:(ci + 1) * abchunk],
                in_=abt_in[ci, :, :])
        abq = abt[:, 0:q]          # queries are the first q key columns

        # value matrices on the gpsimd queue, fine-grained and interleaved
        # so the first key blocks' stationaries land just after exp0
        for vi in range(nvch):
            b0 = vi * vchunk
            b1 = b0 + vchunk
            nc.gpsimd.dma_start(out=mo_t[:, b0:b1, :], in_=mo[vi, :, :])
            nc.gpsimd.dma_start(out=pv_t[:, b0:b1, :], in_=pv[vi, :, :])

        # ---- main loop: all q (both 512-halves) in flight at once.
        # Per key block kb: one stationary load of abt_k shared by the two
        # score matmuls (q halves), ONE wide [128, q] exp, value matmuls two
        # key blocks behind (their es-ready semaphores are then already
        # satisfied when the weight loads issue).
        # PSUM budget: scores [128,1024]x2bufs = 4 banks, psA/psB = 4 banks.
        assert q == 2 * qblk
        # one accumulator tile (= one PSUM bank / zero-region) per q half
        psA = [apool.tile([65, qblk], F32, name=f"psA{j}", tag=f"psA{j}")
               for j in range(2)]
        psB = [apool.tile([64, qblk], F32, name=f"psB{j}", tag=f"psB{j}")
               for j in range(2)]

        def value_mms(es, kb, first, last):
            for m_t, ps in ((mo_t, psA), (pv_t, psB)):
                for j in range(2):
                    nc.tensor.matmul(
                        out=ps[j][:, :],
                        lhsT=m_t[:, kb, :],
                        rhs=es[:, j * qblk:(j + 1) * qblk],
                        start=first, stop=last,
                    )

        es_hist = []
        for kb in range(kblocks):
            ss = spool.tile([128, q], F32)
            for j in range(2):
                nc.tensor.matmul(
                    out=ss[:, j * qblk:(j + 1) * qblk],
                    lhsT=abt[:, kb * 128:(kb + 1) * 128],
                    rhs=abq[:, j * qblk:(j + 1) * qblk],
                    start=True, stop=True,
                )
            es = epool.tile([128, q], F32R)
            nc.scalar.activation(
                es[:, :], ss[:, :], mybir.ActivationFunctionType.Exp,
            )
            if len(es_hist) >= 2:
                value_mms(es_hist[-2], kb - 2, first=(kb == 2), last=False)
            es_hist.append(es)
        value_mms(es_hist[-2], kblocks - 2, first=False, last=False)
        value_mms(es_hist[-1], kblocks - 1, first=False, last=True)

        # outputs: PSUM -> SBUF (DVE) -> DRAM
        for j in range(2):
            qsl = slice(j * qblk, (j + 1) * qblk)
            oA = opool.tile([65, qblk], F32, tag="oA")
            nc.vector.tensor_copy(oA[:, :], psA[j][:, :])
            nc.sync.dma_start(out=om[:, qsl], in_=oA[:, :])
            oB = opool.tile([64, qblk], F32, tag="oB")
            nc.vector.tensor_copy(oB[:, :], psB[j][:, :])
            nc.sync.dma_start(out=op[:, qsl], in_=oB[:, :])

    nc.compile()
    return nc


def make_inputs(mag, phase, n_cores=8):
    """Host-side sharding/layout prep -> per-core (key-rotated) input maps."""
    n, d = mag.shape
    q = n // n_cores
    kblocks = n // 128
    mag = np.ascontiguousarray(mag, dtype=np.float32)
    phase = np.ascontiguousarray(phase, dtype=np.float32)

    a = mag * np.cos(phase)
    b = mag * np.sin(phase)
    abt_g = np.concatenate([a.T, b.T], axis=0).astype(np.float32)  # [128, n]
    mo_nat = np.concatenate([mag, np.ones((n, 1), np.float32)], axis=1)

    abchunk = min(1024, n)
    nabch = n // abchunk
    vchunk = max(1, kblocks // 16)
    nvch = kblocks // vchunk

    def chunk_tr(x):  # [128, n] -> [nabch, 128, abchunk] chunk-major
        return np.ascontiguousarray(
            x.reshape(128, nabch, abchunk).transpose(1, 0, 2))

    def tile_nat(x):  # [n, m] -> [nvch, 128, vchunk*m] chunk-major
        m = x.shape[1]
        y = x.reshape(nvch, vchunk, 128, m).transpose(0, 2, 1, 3)
        return np.ascontiguousarray(y.reshape(nvch, 128, vchunk * m))

    in_maps = []
    for c in range(n_cores):
        r = c * q
        in_maps.append({
            "abt": chunk_tr(np.roll(abt_g, -r, axis=1)),
            "mo": tile_nat(np.roll(mo_nat, -r, axis=0)),
            "pv": tile_nat(np.roll(phase, -r, axis=0)),
        })
    return in_maps


def gather_outputs(results, n, d, n_cores=8):
    """Per-core [65,q]/[64,q] transposed unnormalized sums -> full outputs."""
    new_mag = np.empty((n, d), np.float32)
    new_phase = np.empty((n, d), np.float32)
    q = n // n_cores
    for c in range(n_cores):
        om = results[c]["om"]          # [65, q]
        op = results[c]["op"]          # [64, q]
        den = om[64:65, :]             # [1, q]
        qsl = slice(c * q, (c + 1) * q)
        new_mag[qsl] = (om[:64, :] / den).T
        new_phase[qsl] = (op / den).T
    return new_mag, new_phase


_PROGRAM_CACHE = {}


def _get_program(n, d, n_cores):
    key = (n, d, n_cores)
    if key not in _PROGRAM_CACHE:
        _PROGRAM_CACHE[key] = build_program(n=n, d=d, n_cores=n_cores)
    return _PROGRAM_CACHE[key]


def kernel(mag, phase):
    mag = np.asarray(mag, dtype=np.float32)
    phase = np.asarray(phase, dtype=np.float32)
    n, d = mag.shape
    n_cores = 8
    nc = _get_program(n, d, n_cores)
    in_maps = make_inputs(mag, phase, n_cores=n_cores)
    res = run_bass_kernel_spmd(nc, in_maps, list(range(n_cores)))
    return gather_outputs(res.results, n, d, n_cores=n_cores)


# revision 37
# speedup vs baseline: 1.3624x; 1.1729x over previous
"""ComplexPolarAttention Trainium2 kernel.

score_ij = sum_d mag_i,d mag_j,d cos(phase_i,d - phase_j,d)
         = a_i . a_j + b_i . b_j          with a = mag*cos(phase), b = mag*sin(phase)
out_mag   = softmax(score, axis=1) @ mag
out_phase = softmax(score, axis=1) @ phase

Strategy (8 NeuronCores, SPMD, no collectives):
  - Rows (queries) sharded; keys replicated. Per-core inputs are ROTATED
    along the key axis so that core c's queries are always columns 0..q of
    its own key panel (softmax over keys is permutation invariant), so the
    query operand is just a slice of the key panel.
  - The packed ab^T = [a|b]^T [128=2D, N] panel (host-prepped layout) fuses
    the two score GEMMs into ONE K=128 fp32r matmul per key block of 128.
  - Scores are computed transposed, S^T[k_blk=128, q] in PSUM (one wide
    [128, 1024] exp per key block amortizes ACT's ~352-cycle instruction
    overhead; scores are bounded by D=64 < 88 so unnormalized exp cannot
    overflow), then used as the MOVING operand of the value matmuls whose
    stationary operands are natural-layout [mag|ones] (the ones column
    yields the softmax denominator for free) and phase.
  - PSUM accumulates the numerators over all 64 key blocks; the final
    divide by the denominator happens on host during the gather.
  - All DRAM inputs are chunk-major so every dma_start reads one fully
    contiguous block; the ab^T chunks ride the sync HWDGE queue, the value
    matrices the gpsimd SWDGE queue, so the k-loop's critical first chunk
    lands as early as possible and later chunks stream in under compute.
"""

import numpy as np
from contextlib import ExitStack

import concourse.bass as bass
import concourse.tile as tile
from concourse import bacc, mybir
from concourse.bass_utils import run_bass_kernel_spmd

F32 = mybir.dt.float32
F32R = mybir.dt.float32r


def build_program(n=8192, d=64, n_cores=8, enable_asserts=False):
    """Build the SPMD Bass program. Every core runs identical IR; per-core
    behavior comes only from per-core (rotated) input data."""
    assert d == 64
    q = n // n_cores            # queries per core
    kblocks = n // 128          # key blocks of 128
    qblk = q // 2               # half processed per matmul (fp32 moving max 512)
    assert qblk <= 512 and n % 128 == 0

    nc = bacc.Bacc(
        "TRN2",
        target_bir_lowering=False,
        debug=False,
        enable_asserts=enable_asserts,
        num_devices=n_cores,
    )

    # ---- DRAM I/O (all per-core arrays rotated so queries = keys[0:q]) ----
    abchunk = min(1024, n)
    nabch = n // abchunk
    vchunk = max(1, kblocks // 16)
    nvch = kblocks // vchunk
    # packed [a|b]^T panel, chunk-major [nabch, 128, abchunk]
    abt_in = nc.dram_tensor("abt", [nabch, 128, abchunk], F32R,
                            kind="ExternalInput").ap()
    # [mag | ones] value matrix, chunk-major [nvch, 128, vchunk*65]
    mo = nc.dram_tensor("mo", [nvch, 128, vchunk * 65], F32R,
                        kind="ExternalInput").ap()
    # phase value matrix, chunk-major [nvch, 128, vchunk*64]
    pv = nc.dram_tensor("pv", [nvch, 128, vchunk * d], F32R,
                        kind="ExternalInput").ap()

    om = nc.dram_tensor("om", [65, q], F32, kind="ExternalOutput").ap()
    op = nc.dram_tensor("op", [d, q], F32, kind="ExternalOutput").ap()

    with tile.TileContext(nc) as tc, ExitStack() as ctx:
        persist = ctx.enter_context(tc.tile_pool(name="persist", bufs=1))
        epool = ctx.enter_context(tc.tile_pool(name="exps", bufs=7))
        opool = ctx.enter_context(tc.tile_pool(name="outs", bufs=2))
        spool = ctx.enter_context(tc.tile_pool(name="scores", bufs=2, space="PSUM"))
        apool = ctx.enter_context(tc.tile_pool(name="accum", bufs=1, space="PSUM"))

        abt = persist.tile([128, n], F32R)       # [a|b]^T for all keys
        mo_t = persist.tile([128, kblocks, 65], F32R)
        pv_t = persist.tile([128, kblocks, d], F32R)

        # ab^T chunks on the sync queue -- chunk 0 (== the query slice)
        # gates the first score matmul and exp
        for ci in range(nabch):
            nc.sync.dma_start(
                out=abt[:, ci * abchunk:(ci + 1) * abchunk],
                in_=abt_in[ci, :, :])
        abq = abt[:, 0:q]          # queries are the first q key columns

        # value matrices on the gpsimd queue, fine-grained and interleaved
        # so the first key blocks' stationaries land just after exp0
        for vi in range(nvch):
            b0 = vi * vchunk
            b1 = b0 + vchunk
            nc.gpsimd.dma_start(out=mo_t[:, b0:b1, :], in_=mo[vi, :, :])
            nc.gpsimd.dma_start(out=pv_t[:, b0:b1, :], in_=pv[vi, :, :])

        # ---- main loop: all q (both 512-halves) in flight at once.
        # Per key block kb: one stationary load of abt_k shared by the two
        # score matmuls (q halves), ONE wide [128, q] exp, value matmuls two
        # key blocks behind (their es-ready semaphores are then already
        # satisfied when the weight loads issue).
        # PSUM budget: scores [128,1024]x2bufs = 4 banks, psA/psB = 4 banks.
        assert q == 2 * qblk
        # one accumulator tile (= one PSUM bank / zero-region) per q half
        psA = [apool.tile([65, qblk], F32, name=f"psA{j}", tag=f"psA{j}")
               for j in range(2)]
        psB = [apool.tile([64, qblk], F32, name=f"psB{j}", tag=f"psB{j}")
               for j in range(2)]

        def value_mms(es, kb, first, last):
            for m_t, ps in ((mo_t, psA), (pv_t, psB)):
                for j in range(2):
                    nc.tensor.matmul(
                        out=ps[j][:, :],
                        lhsT=m_t[:, kb, :],
                        rhs=es[:, j * qblk:(j + 1) * qblk],
                        start=first, stop=last,
                    )

        # warm the PE clock (HAM) during the head DMA window: ~3.4us of
        # junk matmuls on zeros so the real stream starts at 2.4 GHz
        wsrc = persist.tile([128, 512], F32)
        nc.vector.memset(wsrc[:, :], 0.0)
        warm = spool.tile([128, q], F32, name="warm", tag="ss")
        wn = min(512, q)
        for _ in range(5):      # fp32 dummies run 4 cyc/row: ~4.3us of PE busy
            nc.tensor.matmul(out=warm[0:16, 0:wn], lhsT=wsrc[:, 0:16],
                             rhs=wsrc[:, 0:wn], start=True, stop=True)

        es_hist = []
        for kb in range(kblocks):
            ss = spool.tile([128, q], F32)
            for j in range(2):
                nc.tensor.matmul(
                    out=ss[:, j * qblk:(j + 1) * qblk],
                    lhsT=abt[:, kb * 128:(kb + 1) * 128],
                    rhs=abq[:, j * qblk:(j + 1) * qblk],
                    start=True, stop=True,
                )
            es = epool.tile([128, q], F32R)
            nc.scalar.activation(
                es[:, :], ss[:, :], mybir.ActivationFunctionType.Exp,
            )
            if len(es_hist) >= 3:
                value_mms(es_hist[-3], kb - 3, first=(kb == 3), last=False)
            es_hist.append(es)
        value_mms(es_hist[-3], kblocks - 3, first=False, last=False)
        value_mms(es_hist[-2], kblocks - 2, first=False, last=False)
        value_mms(es_hist[-1], kblocks - 1, first=False, last=True)

        # outputs: PSUM -> SBUF (DVE) -> DRAM
        for j in range(2):
            qsl = slice(j * qblk, (j + 1) * qblk)
            oA = opool.tile([65, qblk], F32, tag="oA")
            nc.vector.tensor_copy(oA[:, :], psA[j][:, :])
            nc.sync.dma_start(out=om[:, qsl], in_=oA[:, :])
            oB = opool.tile([64, qblk], F32, tag="oB")
            nc.vector.tensor_copy(oB[:, :], psB[j][:, :])
            nc.sync.dma_start(out=op[:, qsl], in_=oB[:, :])

    nc.compile()
    return nc


def make_inputs(mag, phase, n_cores=8):
    """Host-side sharding/layout prep -> per-core (key-rotated) input maps."""
    n, d = mag.shape
    q = n // n_cores
    kblocks = n // 128
    mag = np.ascontiguousarray(mag, dtype=np.float32)
    phase = np.ascontiguousarray(phase, dtype=np.float32)

    a = mag * np.cos(phase)
    b = mag * np.sin(phase)
    abt_g = np.concatenate([a.T, b.T], axis=0).astype(np.float32)  # [128, n]
    mo_nat = np.concatenate([mag, np.ones((n, 1), np.float32)], axis=1)

    abchunk = min(1024, n)
    nabch = n // abchunk
    vchunk = max(1, kblocks // 16)
    nvch = kblocks // vchunk

    def chunk_tr(x):  # [128, n] -> [nabch, 128, abchunk] chunk-major
        return np.ascontiguousarray(
            x.reshape(128, nabch, abchunk).transpose(1, 0, 2))

    def tile_nat(x):  # [n, m] -> [nvch, 128, vchunk*m] chunk-major
        m = x.shape[1]
        y = x.reshape(nvch, vchunk, 128, m).transpose(0, 2, 1, 3)
        return np.ascontiguousarray(y.reshape(nvch, 128, vchunk * m))

    in_maps = []
    for c in range(n_cores):
        r = c * q
        in_maps.append({
            "abt": chunk_tr(np.roll(abt_g, -r, axis=1)),
            "mo": tile_nat(np.roll(mo_nat, -r, axis=0)),
            "pv": tile_nat(np.roll(phase, -r, axis=0)),
        })
    return in_maps


def gather_outputs(results, n, d, n_cores=8):
    """Per-core [65,q]/[64,q] transposed unnormalized sums -> full outputs."""
    new_mag = np.empty((n, d), np.float32)
    new_phase = np.empty((n, d), np.float32)
    q = n // n_cores
    for c in range(n_cores):
        om = results[c]["om"]          # [65, q]
        op = results[c]["op"]          # [64, q]
        den = om[64:65, :]             # [1, q]
        qsl = slice(c * q, (c + 1) * q)
        new_mag[qsl] = (om[:64, :] / den).T
        new_phase[qsl] = (op / den).T
    return new_mag, new_phase


_PROGRAM_CACHE = {}


def _get_program(n, d, n_cores):
    key = (n, d, n_cores)
    if key not in _PROGRAM_CACHE:
        _PROGRAM_CACHE[key] = build_program(n=n, d=d, n_cores=n_cores)
    return _PROGRAM_CACHE[key]


def kernel(mag, phase):
    mag = np.asarray(mag, dtype=np.float32)
    phase = np.asarray(phase, dtype=np.float32)
    n, d = mag.shape
    n_cores = 8
    nc = _get_program(n, d, n_cores)
    in_maps = make_inputs(mag, phase, n_cores=n_cores)
    res = run_bass_kernel_spmd(nc, in_maps, list(range(n_cores)))
    return gather_outputs(res.results, n, d, n_cores=n_cores)
